# revision 1
# baseline (speedup 1.0000x reference)
"""Causal self-attention block (LN -> QKV -> causal attention -> out-proj)
on 8 Trainium2 NeuronCores.

Sharding: core = 2*batch + head_group. Each core handles one batch element
(S=2048 tokens) and 8 of the 16 heads (tensor-parallel split of w_qkv along
the head axis and w_out along its input dim). The two partial outputs per
batch are summed on the host (the all-reduce of the sharding hint).

Device kernel layout strategy (per core):
  - LayerNorm in natural layout [s, d], then PE-transpose to xnT [d, s]
    (contraction dim must sit on partitions for matmuls).
  - QKV projection computes q^T/k^T in [head_dim, s] layout directly and V in
    natural [s, head_dim] layout, so causal attention needs no further
    transposes: scores are computed transposed, ST[k, q] = k . q, softmax'd
    along the partition-free axis via exp + a ones-column appended to V
    (the PV matmul then yields both y^T and the softmax row-sums).
  - ln_scale/ln_bias/b_qkv/softmax-scale/b_out are all folded into the
    weights on the host; matmuls run as float32r (FP22, full PE rate).
"""

import os

# the device path runs through jax's axon PJRT plugin; make sure a
# pre-set JAX_PLATFORMS doesn't hide it (unset = all plugins load)
_jp = os.environ.get("JAX_PLATFORMS")
if _jp and "axon" not in _jp:
    os.environ["JAX_PLATFORMS"] = f"axon,{_jp}"

import numpy as np

import concourse.bass as bass
import concourse.mybir as mybir
import concourse.tile as tile
from concourse import bacc
from concourse.bass_utils import run_bass_kernel_spmd
from concourse.masks import make_identity

B, S, D, H, HD = 4, 2048, 1024, 16, 64
HL = H // 2          # heads per core (local)
NCH = D // 128       # 8 contraction chunks
NSB = S // 128       # 16 s-blocks
NQS = S // 512       # 4 q-superblocks
NEG = -1.0e38
LN_EPS = 1e-6

f32 = mybir.dt.float32
f32r = mybir.dt.float32r

_cache = {}


def build_program():
    nc = bacc.Bacc()

    x_d = nc.declare_dram_parameter("x", [S, D], f32, isOutput=False)
    wqk_d = nc.declare_dram_parameter("wqk", [NCH, 128, 1024], f32r, isOutput=False)
    wv_d = nc.declare_dram_parameter("wv", [NCH, 128, 512], f32r, isOutput=False)
    bqk_d = nc.declare_dram_parameter("bqk", [128, 2, 4], f32, isOutput=False)
    bv1_d = nc.declare_dram_parameter("bv1", [1, 512], f32r, isOutput=False)
    vones_d = nc.declare_dram_parameter("vones", [1, 128], f32r, isOutput=False)
    wout_d = nc.declare_dram_parameter("wout", [4, 128, 1024], f32r, isOutput=False)
    out_d = nc.declare_dram_parameter("out", [S, D], f32, isOutput=True)

    with tile.TileContext(nc, pool_alloc_mode="queue") as tc:
        with (
            tc.tile_pool(name="singles", bufs=1) as singles,
            tc.tile_pool(name="qkT", bufs=1) as qkTp,
            tc.tile_pool(name="vpool", bufs=1) as vpool,
            tc.tile_pool(name="pscm", bufs=1, space="PSUM") as pscm,
        ):
            # ---- constants ----
            ident = singles.tile([128, 128], f32)
            make_identity(nc, ident)
            identb = singles.tile([128, 128], mybir.dt.bfloat16)
            make_identity(nc, identb)
            maskTb = singles.tile([128, 128], mybir.dt.bfloat16)
            nc.gpsimd.memset(maskTb, 0.0)
            nc.gpsimd.affine_select(
                out=maskTb, in_=maskTb,
                compare_op=mybir.AluOpType.is_ge,
                fill=NEG, base=0,
                pattern=[[1, 128]], channel_multiplier=-1,
            )
            eps_t = singles.tile([128, 1], f32)
            nc.vector.memset(eps_t, LN_EPS)
            bqk_t = singles.tile([128, 2, 4], f32)
            nc.sync.dma_start(out=bqk_t, in_=bqk_d[:, :, :])
            bv1_t = singles.tile([1, 512], f32r)
            nc.sync.dma_start(out=bv1_t, in_=bv1_d[:, :])
            vones_t = singles.tile([1, 128], f32r)
            nc.sync.dma_start(out=vones_t, in_=vones_d[:, :])

            # ---- persistent activations ----
            qT = qkTp.tile([128, 4, S], f32r)   # [pair-row, pair, s]
            kT = qkTp.tile([128, 4, S], f32r)
            # V'' [s-row, s-block, head, 65] (col 64 = ones)
            vpp = vpool.tile([128, NSB, HL, HD + 1], f32r)
            nc.gpsimd.memset(vpp[:, :, :, HD : HD + 1].bitcast(f32), 1.0)

            # ================= Phase A: LayerNorm + transpose =================
            with tc.tile_pool(name="xnT", bufs=1) as xnTp:
                xnT = xnTp.tile([128, NCH, S], f32r)
                with (
                    tc.tile_pool(name="atmp", bufs=5) as atmp,
                    tc.tile_pool(name="astat", bufs=8) as astat,
                ):
                    for i in range(NSB):
                        x_t = atmp.tile([128, D], f32, tag="x")
                        nc.sync.dma_start(out=x_t, in_=x_d[i * 128 : (i + 1) * 128, :])
                        stats = astat.tile([128, 2, 6], f32, tag="stats")
                        nc.vector.bn_stats(out=stats[:, 0, :], in_=x_t[:, 0:512])
                        nc.vector.bn_stats(out=stats[:, 1, :], in_=x_t[:, 512:1024])
                        mv = astat.tile([128, 2], f32, tag="mv")
                        nc.vector.bn_aggr(out=mv, in_=stats)
                        std_t = astat.tile([128, 1], f32, tag="std")
                        nc.scalar.activation(
                            out=std_t, in_=mv[:, 1:2],
                            func=mybir.ActivationFunctionType.Sqrt,
                            bias=eps_t, scale=1.0,
                        )
                        rstd_t = astat.tile([128, 1], f32, tag="rstd")
                        nc.vector.reciprocal(out=rstd_t, in_=std_t)
                        xn_t = atmp.tile([128, D], f32, tag="xn")
                        nc.vector.tensor_scalar(
                            out=xn_t, in0=x_t,
                            scalar1=mv[:, 0:1], scalar2=rstd_t,
                            op0=mybir.AluOpType.subtract, op1=mybir.AluOpType.mult,
                        )
                        for c4 in range(0, NCH, 4):
                            pst = pscm.tile([128, 4, 128], f32, tag="yt", bufs=4)
                            for c in range(c4, c4 + 4):
                                nc.tensor.transpose(
                                    pst[:, c - c4, :],
                                    xn_t[:, c * 128 : (c + 1) * 128],
                                    ident,
                                )
                            nc.scalar.activation(
                                out=xnT[:, c4 : c4 + 4, i * 128 : (i + 1) * 128],
                                in_=pst,
                                func=mybir.ActivationFunctionType.Copy,
                            )

                # ================= Phase B: QKV projection =================
                with (
                    tc.tile_pool(name="wqk", bufs=2) as wqkp,
                    tc.tile_pool(name="wvp", bufs=1) as wvp,
                ):
                    def emit_qk(t, p):
                        fb = t * 4 + p
                        w_t = wqkp.tile([128, NCH, 128], f32r, tag="wqk",
                                        name=f"wqk_{t}_{p}")
                        nc.sync.dma_start(
                            out=w_t,
                            in_=wqk_d[:, :, fb * 128 : (fb + 1) * 128].rearrange(
                                "c d f -> d c f"
                            ),
                        )
                        dest = qT if t == 0 else kT
                        for sb in range(NQS):
                            ps = pscm.tile([128, 512], f32, tag="st", bufs=2,
                                           name=f"psqk_{t}_{p}_{sb}")
                            for c in range(NCH):
                                nc.tensor.matmul(
                                    ps,
                                    w_t[:, c, :],
                                    xnT[:, c, sb * 512 : (sb + 1) * 512],
                                    start=(c == 0),
                                    stop=(c == NCH - 1),
                                )
                            nc.vector.tensor_scalar_add(
                                out=dest[:, p, sb * 512 : (sb + 1) * 512],
                                in0=ps,
                                scalar1=bqk_t[:, t, p : p + 1],
                            )

                    def emit_v():
                        wv_t = wvp.tile([128, NCH, 512], f32r)
                        for c in range(NCH):
                            nc.sync.dma_start(out=wv_t[:, c, :], in_=wv_d[c, :, :])
                        for i in range(NSB):
                            psv = pscm.tile([128, 512], f32, tag="st", bufs=2,
                                            name=f"psv_{i}")
                            for c in range(NCH):
                                nc.tensor.matmul(
                                    psv,
                                    xnT[:, c, i * 128 : (i + 1) * 128],
                                    wv_t[:, c, :],
                                    start=(c == 0),
                                    stop=False,
                                )
                            # += ones[s] x bv  (rank-1 bias update)
                            nc.tensor.matmul(
                                psv, vones_t, bv1_t, start=False, stop=True,
                            )
                            nc.vector.tensor_copy(
                                vpp[:, i, :, 0:HD],
                                psv.rearrange("p (h v) -> p h v", v=HD),
                            )

                    # pair 0 first, then V, so attention on heads 0/1 can
                    # start while the rest of the projection still runs
                    emit_qk(0, 0)
                    emit_qk(1, 0)
                    emit_v()
                    for p in range(1, 4):
                        emit_qk(0, p)
                        emit_qk(1, p)

            # ================= Phase C: causal attention =================
            with tc.tile_pool(name="ytall", bufs=1) as ytallp:
                ytall = ytallp.tile([128, 4, S], f32r)  # [pair-row, pair, s]
                with (
                    tc.tile_pool(name="ptp", bufs=6) as ptp,
                    tc.tile_pool(name="ctmp", bufs=4) as ctmp,
                    tc.tile_pool(name="dscr", bufs=8, space="DRAM") as dscr,
                    tc.tile_pool(name="woutp", bufs=1) as woutp,
                    tc.tile_pool(name="ypool", bufs=3) as ypool,
                ):
                    wout_t = woutp.tile([128, 4, 1024], f32r)
                    for c in range(4):
                        nc.sync.dma_start(out=wout_t[:, c, :], in_=wout_d[c, :, :])
                    def emit_outproj(i):
                        y_t = ypool.tile([128, 1024], f32, tag="y",
                                         name=f"y_{i}")
                        for nh in range(2):
                            # alternate tags: the yt slots are idle during
                            # the output projection, use them for depth
                            pso = pscm.tile([128, 512], f32,
                                            tag=("st" if nh == 0 else "yt"),
                                            bufs=(2 if nh == 0 else 4),
                                            name=f"pso_{i}_{nh}")
                            for c in range(4):
                                nc.tensor.matmul(
                                    pso,
                                    ytall[:, c, i * 128 : (i + 1) * 128],
                                    wout_t[:, c, nh * 512 : (nh + 1) * 512],
                                    start=(c == 0),
                                    stop=(c == 3),
                                )
                            nc.vector.tensor_copy(
                                y_t[:, nh * 512 : (nh + 1) * 512], pso
                            )
                        nc.sync.dma_start(
                            out=out_d[i * 128 : (i + 1) * 128, :], in_=y_t
                        )

                    for sb in range(NQS):
                        for p in range(4):
                            # the pair's two heads (PE rows 0:64 / 64:128)
                            # run as adjacent matmuls -> concurrent row-groups
                            q0 = sb * 512
                            jmax = 4 * sb + 3
                            yts = [
                                pscm.tile([HD + 1, 512], f32, tag="yt",
                                          bufs=4, name=f"yt_{2 * p + hf}_{sb}")
                                for hf in range(2)
                            ]
                            for j in range(jmax + 1):
                                r = max(0, j - 4 * sb)
                                diag = j >= 4 * sb
                                L = 512 - 128 * r
                                st = pscm.tile([128, 1024], f32, tag="st",
                                               bufs=2, name=f"st_{p}_{sb}_{j}")
                                pt = ptp.tile([128, 1024], f32r, tag="pt")
                                for hf in range(2):
                                    rows = slice(hf * HD, (hf + 1) * HD)
                                    # hf0 packs left in bank 0; hf1 must stay
                                    # bank-aligned at 512 (matmul outputs
                                    # cannot cross a PSUM bank boundary)
                                    lo = hf * 512
                                    nc.tensor.matmul(
                                        st[:, lo : lo + L],
                                        kT[rows, p, j * 128 : (j + 1) * 128],
                                        qT[rows, p, q0 + r * 128 : q0 + 512],
                                        start=True, stop=not diag,
                                    )
                                if diag:
                                    # causal mask folded in on the PE:
                                    # st[diag] += I.T @ maskT
                                    for hf in range(2):
                                        nc.tensor.matmul(
                                            st[:, hf * 512 : hf * 512 + 128],
                                            identb,
                                            maskTb,
                                            start=False, stop=True,
                                        )
                                # one wide exp across both heads (for r>0 the
                                # [L:512) strip is unread garbage)
                                nc.scalar.activation(
                                    out=pt[:, 0 : 512 + L],
                                    in_=st[:, 0 : 512 + L],
                                    func=mybir.ActivationFunctionType.Exp,
                                )
                                for hf in range(2):
                                    nc.tensor.matmul(
                                        yts[hf][:, r * 128 : 512],
                                        vpp[:, j, 2 * p + hf, :],
                                        pt[:, hf * 512 : hf * 512 + L],
                                        start=(j == 0),
                                        stop=(j == jmax),
                                    )
                            # per-superblock softmax normalization epilogue
                            for hf in range(2):
                                rows = slice(hf * HD, (hf + 1) * HD)
                                yt = yts[hf]
                                ssum = ctmp.tile([1, 512], f32, tag="ssum")
                                nc.vector.tensor_copy(ssum, yt[HD : HD + 1, :])
                                dsum = dscr.tile([512], f32, tag="dsum")
                                nc.sync.dma_start(out=dsum, in_=ssum)
                                sums4 = ctmp.tile([4, 128], f32, tag="sums4")
                                nc.sync.dma_start(
                                    out=sums4,
                                    in_=dsum.rearrange("(a b) -> a b", b=128),
                                )
                                sinv4 = ctmp.tile([4, 128], f32, tag="sinv4")
                                nc.vector.reciprocal(out=sinv4, in_=sums4)
                                dsinv = dscr.tile([512], f32, tag="dsinv")
                                nc.sync.dma_start(
                                    out=dsinv.rearrange("(a b) -> a b", b=128),
                                    in_=sinv4,
                                )
                                src = dsinv[:]
                                bcast = bass.AP(
                                    tensor=src.tensor,
                                    offset=src.offset,
                                    ap=[[0, HD]] + list(src.ap),
                                )
                                binv = ctmp.tile([HD, 512], f32, tag="binv")
                                nc.sync.dma_start(out=binv, in_=bcast)
                                nc.vector.tensor_mul(
                                    out=ytall[rows, p, q0 : q0 + 512],
                                    in0=yt[0:HD, :],
                                    in1=binv,
                                )

                    for i in range(NSB):
                        emit_outproj(i)

    nc.finalize()
    return nc


def _prep_core_inputs(x, ln_scale, ln_bias, w_qkv, b_qkv, w_out):
    """Host-side folding + per-core input maps."""
    scale = np.float32(HD ** -0.5)
    # qkv = xn@W + b_qkv, xn = z*ln_scale + ln_bias  =>  z @ (ln_scale*W) + (ln_bias@W + b_qkv)
    b_eff = b_qkv + np.einsum(
        "d,dhf->hf", ln_bias.astype(np.float64), w_qkv.astype(np.float64)
    ).astype(np.float32)
    w_eff = ln_scale[:, None, None] * w_qkv
    wq = w_eff[:, :, 0:64] * scale
    wk = w_eff[:, :, 64:128]
    wv = w_eff[:, :, 128:192]
    bq = b_eff[:, 0:64] * scale
    bk = b_eff[:, 64:128]
    bv = b_eff[:, 128:192]

    in_maps = []
    for core in range(8):
        b, g = core // 2, core % 2
        hsel = slice(g * HL, (g + 1) * HL)
        # [D, 4 pairs, 128] with head 2p in rows 0:64, head 2p+1 in 64:128
        qp = wq[:, hsel].reshape(D, 4, 128)
        kp = wk[:, hsel].reshape(D, 4, 128)
        wqk = np.concatenate(
            [qp.reshape(D, 512), kp.reshape(D, 512)], axis=1
        ).reshape(NCH, 128, 1024)
        wv_g = np.ascontiguousarray(wv[:, hsel].reshape(D, 512)).reshape(
            NCH, 128, 512
        )
        bq_p = bq[hsel].reshape(4, 128)
        bk_p = bk[hsel].reshape(4, 128)
        bqk = np.ascontiguousarray(
            np.stack([bq_p, bk_p], axis=0).transpose(2, 0, 1)
        )
        bv1 = np.ascontiguousarray(bv[hsel].reshape(1, 512))
        wout = np.ascontiguousarray(
            w_out[g * 512 : (g + 1) * 512, :].reshape(4, 128, 1024)
        )
        in_maps.append(
            {
                "x": np.ascontiguousarray(x[b]),
                "wqk": np.ascontiguousarray(wqk),
                "wv": wv_g,
                "bqk": bqk,
                "bv1": bv1,
                "vones": np.ones((1, 128), np.float32),
                "wout": wout,
            }
        )
    return in_maps


def kernel(x, mask, ln_scale, ln_bias, w_qkv, b_qkv, w_out, b_out, **run_kwargs):
    x = np.asarray(x, np.float32)
    ln_scale = np.asarray(ln_scale, np.float32)
    ln_bias = np.asarray(ln_bias, np.float32)
    w_qkv = np.asarray(w_qkv, np.float32)
    b_qkv = np.asarray(b_qkv, np.float32)
    w_out = np.asarray(w_out, np.float32)
    b_out = np.asarray(b_out, np.float32)
    if "nc" not in _cache:
        _cache["nc"] = build_program()
    nc = _cache["nc"]
    in_maps = _prep_core_inputs(x, ln_scale, ln_bias, w_qkv, b_qkv, w_out)
    res = run_bass_kernel_spmd(nc, in_maps, list(range(8)), **run_kwargs)
    _cache["last_result"] = res
    out = np.empty((B, S, D), np.float32)
    for b in range(B):
        out[b] = res.results[2 * b]["out"] + res.results[2 * b + 1]["out"]
    out += np.asarray(b_out)[None, None, :]
    return out

